# revision 1
# baseline (speedup 1.0000x reference)
"""AGNN (3-layer) Trainium2 kernel.

Strategy (see spec sharding_hint): nodes are partitioned across the 8
NeuronCores by destination (graph/data parallel). Edges are routed to the
core owning the destination node. Per core, destination nodes are grouped
into degree classes (in-degree padded to a multiple of 4, capped at 128);
each 128-slot "block" holds m = 128//K nodes' padded edge lists stacked on
partitions. Per-destination segment softmax and the weighted aggregation
are computed with fixed per-class 0/1 indicator matmuls on the tensor
engine (contraction over partitions), the per-edge cosine logits with the
vector engine, exp on the scalar engine.

The device kernel is a pure streaming+compute NEFF (one compile, one run
per layer). Source-node feature rows for every edge slot are gathered on
the host between layers (the normalized features table is small and the
hardware available here has no fast data-dependent gather primitive — the
extended-ucode DMA gather is absent and indirect DMA runs at ~265ns/row,
measured, which is orders of magnitude off the roofline).
"""

import math
import os
import numpy as np
from contextlib import ExitStack

N_NODES = 100000
D = 32
N_CORES = 8
NPC = N_NODES // N_CORES          # nodes per core
EPS = 1e-12
PAY = D + 1                        # slot payload: xn (32) + norm (1)
CB = 32                            # blocks per compute chunk
SUBRUN = 116                       # max blocks per softmax subrun
Y_ON_GPSIMD = True                 # run the weighted-value pass on the Pool engine
_NEFF_CACHE = {}


# ----------------------------------------------------------------------------
# host-side graph preprocessing (layer-invariant)
# ----------------------------------------------------------------------------

class Plan:
    pass


def build_plan(src, dst):
    """src/dst: int64 [E_tot] edge endpoints including self loops."""
    deg = np.bincount(dst, minlength=N_NODES)
    assert deg.max() <= 128, f"max in-degree {deg.max()} > 128 unsupported"
    K_of_node = 4 * np.ceil(deg / 4).astype(np.int64)
    K_of_node = np.maximum(K_of_node, 4)

    # per-core: sort local nodes by K descending (stable)
    plan = Plan()
    plan.core_nodes = []        # per core: original node ids in sorted order
    plan.core_Ks = []
    for c in range(N_CORES):
        nodes = np.arange(c * NPC, (c + 1) * NPC)
        order = np.argsort(-K_of_node[nodes], kind="stable")
        plan.core_nodes.append(nodes[order])
        plan.core_Ks.append(K_of_node[nodes[order]])

    # class structure equalized across cores
    all_K = sorted(set(int(k) for c in range(N_CORES) for k in plan.core_Ks[c]),
                   reverse=True)
    plan.classes = []           # list of (K, m, nblk)
    for K in all_K:
        m = 128 // K
        nblk = 0
        for c in range(N_CORES):
            nk = int((plan.core_Ks[c] == K).sum())
            nblk = max(nblk, (nk + m - 1) // m)
        plan.classes.append((K, m, nblk))
    plan.NBLK = sum(nblk for _, _, nblk in plan.classes)
    plan.ARRW = plan.NBLK * D

    # per class: column offset in the arrangement tensors
    plan.class_arr_off = []
    off = 0
    for (K, m, nblk) in plan.classes:
        plan.class_arr_off.append(off)
        off += nblk * D
    plan.class_blk_off = []
    off = 0
    for (K, m, nblk) in plan.classes:
        plan.class_blk_off.append(off)
        off += nblk

    # per-core slot->src map [128, NBLK], mask [128, NBLK],
    # node placement map (class arrangement)
    # edges grouped by dst: CSR over sorted nodes
    e_order = np.argsort(dst, kind="stable")
    src_by_dst = src[e_order]
    row_start = np.zeros(N_NODES + 1, dtype=np.int64)
    np.cumsum(deg, out=row_start[1:])

    plan.slot_src = np.zeros((N_CORES, 128, plan.NBLK), dtype=np.int32)
    plan.mask = np.zeros((N_CORES, 128, plan.NBLK), dtype=np.float32)
    # arrangement: for core c, class ci, block b, j -> original node id (or -1)
    plan.arr_node = np.full((N_CORES, 32, plan.NBLK), -1, dtype=np.int64)

    for c in range(N_CORES):
        Ks = plan.core_Ks[c]
        nodes_sorted = plan.core_nodes[c]
        pos = 0
        for ci, (K, m, nblk) in enumerate(plan.classes):
            nk = int((Ks == K).sum())
            cls_nodes = nodes_sorted[pos:pos + nk]
            pos += nk
            b0 = plan.class_blk_off[ci]
            for j_global in range(nk):
                b = j_global // m
                j = j_global % m
                node = cls_nodes[j_global]
                plan.arr_node[c, j, b0 + b] = node
                d0 = deg[node]
                p0 = j * K
                ss = src_by_dst[row_start[node]:row_start[node] + d0]
                plan.slot_src[c, p0:p0 + d0, b0 + b] = ss
                plan.mask[c, p0:p0 + d0, b0 + b] = 1.0
    return plan


def host_normalize(x):
    nrm = np.sqrt((x.astype(np.float64) ** 2).sum(axis=1))
    nrm = np.maximum(nrm, EPS).astype(np.float32)
    xn = (x / nrm[:, None]).astype(np.float32)
    return xn, nrm


def host_layer_inputs(plan, x_full, beta):
    """Build per-core device inputs for one layer from the full node features."""
    xn, nrm = host_normalize(x_full)
    ins = []
    for c in range(N_CORES):
        ss = plan.slot_src[c]                       # [128, NBLK]
        xsl = np.empty((128, plan.NBLK, PAY), dtype=np.float32)
        xsl[:, :, :D] = xn[ss]
        xsl[:, :, D] = nrm[ss]
        xarr = np.zeros((32, plan.ARRW), dtype=np.float32)
        for ci, (K, m, nblk) in enumerate(plan.classes):
            a0 = plan.class_arr_off[ci]
            b0 = plan.class_blk_off[ci]
            nodes = plan.arr_node[c, :m, b0:b0 + nblk]     # [m, nblk]
            valid = nodes >= 0
            xa = np.zeros((m, nblk, D), dtype=np.float32)
            xa[valid] = beta * xn[nodes[valid]]
            xarr[:m, a0:a0 + nblk * D] = xa.reshape(m, nblk * D)
        ins.append({
            "xsl": np.ascontiguousarray(xsl.reshape(128, plan.NBLK * PAY)),
            "xarr": xarr,
            "mask": np.ascontiguousarray(plan.mask[c]),
        })
    return ins


def host_collect_output(plan, oarrs):
    """oarrs: per-core [32, ARRW] aggregation results -> full [N, D]."""
    out = np.zeros((N_NODES, D), dtype=np.float32)
    for c in range(N_CORES):
        oa = oarrs[c]
        for ci, (K, m, nblk) in enumerate(plan.classes):
            a0 = plan.class_arr_off[ci]
            b0 = plan.class_blk_off[ci]
            nodes = plan.arr_node[c, :m, b0:b0 + nblk]     # [m, nblk]
            vals = oa[:m, a0:a0 + nblk * D].reshape(m, nblk, D)
            valid = nodes >= 0
            out[nodes[valid]] = vals[valid]
    return out


# ----------------------------------------------------------------------------
# device kernel
# ----------------------------------------------------------------------------

def host_indicators(plan):
    """Packed per-class indicator matrices (identical for every core)."""
    uniq = []
    seen = set()
    for (K, m, nblk) in plan.classes:
        if (K, m) not in seen:
            seen.add((K, m))
            uniq.append((K, m))
    plan.ind_uniq = uniq
    indk = np.zeros((128, sum(m for _, m in uniq)), dtype=np.float32)
    indkt = np.zeros((32, 128 * len(uniq)), dtype=np.float32)
    plan.ind_off = {}
    off = 0
    for i, (K, m) in enumerate(uniq):
        plan.ind_off[(K, m)] = (off, i)
        p = np.arange(128)
        sel = (p // K) < m
        indk[sel, off + (p // K)[sel]] = 1.0
        indkt[:m, i * 128:(i + 1) * 128] = indk[:, off:off + m].T
        off += m
    plan.indk_w = indk.shape[1]
    plan.n_ind = len(uniq)
    return indk, indkt


def build_nc(plan):
    import concourse.bass as bass
    import concourse.tile as tile
    from concourse import bacc, mybir

    f32 = mybir.dt.float32
    nc = bacc.Bacc("TRN2", target_bir_lowering=False, debug=False)
    xsl_d = nc.declare_dram_parameter("xsl", [128, plan.NBLK * PAY], f32, isOutput=False)
    xarr_d = nc.declare_dram_parameter("xarr", [32, plan.ARRW], f32, isOutput=False)
    mask_d = nc.declare_dram_parameter("mask", [128, plan.NBLK], f32, isOutput=False)
    indk_d = nc.declare_dram_parameter("indk", [128, plan.indk_w], f32, isOutput=False)
    indkt_d = nc.declare_dram_parameter("indkt", [32, 128 * plan.n_ind], f32, isOutput=False)
    oarr_d = nc.declare_dram_parameter("oarr", [32, plan.ARRW], f32, isOutput=True)

    # subrun schedule: (class_idx, blk_off_in_class, nblk_sub)
    subruns = []
    for ci, (K, m, nblk) in enumerate(plan.classes):
        b = 0
        while b < nblk:
            n = min(SUBRUN, nblk - b)
            subruns.append((ci, b, n))
            b += n

    with tile.TileContext(nc) as tc, ExitStack() as ctx:
        const = ctx.enter_context(tc.tile_pool(name="const", bufs=1))
        xpool = ctx.enter_context(tc.tile_pool(name="xsl", bufs=5))
        apool = ctx.enter_context(tc.tile_pool(name="arr", bufs=2))
        wpool = ctx.enter_context(tc.tile_pool(name="work", bufs=4))
        spool = ctx.enter_context(tc.tile_pool(name="small", bufs=4))
        opool = ctx.enter_context(tc.tile_pool(name="outp", bufs=2))
        ps_x = ctx.enter_context(tc.tile_pool(name="psx", bufs=2, space="PSUM"))
        ps_s = ctx.enter_context(tc.tile_pool(name="pss", bufs=2, space="PSUM"))
        ps_a = ctx.enter_context(tc.tile_pool(name="psa", bufs=2, space="PSUM"))

        # resident constants
        mask_sb = const.tile([128, plan.NBLK], f32)
        nc.sync.dma_start(out=mask_sb[:], in_=mask_d[:])
        indk_sb = const.tile([128, plan.indk_w], f32)
        nc.sync.dma_start(out=indk_sb[:], in_=indk_d[:])
        indkt_sb = const.tile([32, 128 * plan.n_ind], f32)
        nc.sync.dma_start(out=indkt_sb[:], in_=indkt_d[:])

        state = {}

        def ctx_of(si):
            (ci, bo, R) = subruns[si]
            K, m, nblk = plan.classes[ci]
            a0 = plan.class_arr_off[ci] + bo * D
            b0 = plan.class_blk_off[ci] + bo
            ioff, iidx = plan.ind_off[(K, m)]
            return (K, m, a0, b0,
                    indk_sb[:, ioff:ioff + m],
                    indkt_sb[:, iidx * 128:(iidx + 1) * 128], R)

        def emit_A(si):
            K, m, a0, b0, indk, indkt, R = ctx_of(si)
            xs = xpool.tile([128, SUBRUN * PAY], f32, tag="xs")
            nc.sync.dma_start(out=xs[:, :R * PAY],
                              in_=xsl_d[:, b0 * PAY:(b0 + R) * PAY])
            xs3 = xs[:, :R * PAY].rearrange("p (b w) -> p b w", b=R, w=PAY)
            xa = apool.tile([32, SUBRUN * D], f32, tag="xa")
            nc.sync.dma_start(out=xa[:m, :R * D], in_=xarr_d[:m, a0:a0 + R * D])
            alpha = spool.tile([128, SUBRUN], f32, tag="alpha")
            CBA = 16
            for q in range((R + CBA - 1) // CBA):
                cb = min(CBA, R - q * CBA)
                xnd = ps_x.tile([128, CBA * D], f32, tag="xnd")
                for h in range(0, cb * D, 512):
                    hw_ = min(512, cb * D - h)
                    nc.tensor.matmul(out=xnd[:, h:h + hw_], lhsT=indkt[:m, :],
                                     rhs=xa[:m, q * CBA * D + h:q * CBA * D + h + hw_],
                                     start=True, stop=True)
                prod = wpool.tile([128, CBA * D], f32, tag="prod")
                nc.vector.tensor_tensor(
                    out=prod[:, :cb * D].rearrange("p (b w) -> p b w", b=cb, w=D),
                    in0=xs3[:, q * CBA:q * CBA + cb, 0:D],
                    in1=xnd[:, :cb * D].rearrange("p (b w) -> p b w", b=cb, w=D),
                    op=mybir.AluOpType.mult)
                nc.vector.tensor_reduce(
                    out=alpha[:, q * CBA:q * CBA + cb],
                    in_=prod[:, :cb * D].rearrange("p (b w) -> p b w", b=cb, w=D),
                    axis=mybir.AxisListType.X, op=mybir.AluOpType.add)
            state[si] = {"xs3": xs3, "alpha": alpha}

        def emit_B(si):
            K, m, a0, b0, indk, indkt, R = ctx_of(si)
            st = state[si]
            alpha, xs3 = st["alpha"], st["xs3"]
            # softmax over segments (global-max-free: |alpha| <= |beta|)
            e = spool.tile([128, SUBRUN], f32, tag="e")
            nc.scalar.activation(e[:, :R], alpha[:, :R],
                                 mybir.ActivationFunctionType.Exp, 0.0, 1.0)
            em = spool.tile([128, SUBRUN], f32, tag="em")
            nc.vector.tensor_tensor(out=em[:, :R], in0=e[:, :R],
                                    in1=mask_sb[:, b0:b0 + R],
                                    op=mybir.AluOpType.mult)
            ss = ps_s.tile([128, SUBRUN], f32, tag="sm")
            nc.tensor.matmul(out=ss[:m, :R], lhsT=indk[:, :m], rhs=em[:, :R],
                             start=True, stop=True)
            ssr = spool.tile([32, SUBRUN], f32, tag="ssr")
            nc.scalar.activation(ssr[:m, :R], ss[:m, :R],
                                 mybir.ActivationFunctionType.Copy, 1e-30, 1.0)
            rs = spool.tile([32, SUBRUN], f32, tag="rs")
            nc.vector.reciprocal(rs[:m, :R], ssr[:m, :R])
            rsrep = ps_s.tile([128, SUBRUN], f32, tag="sm")
            nc.tensor.matmul(out=rsrep[:, :R], lhsT=indkt[:m, :], rhs=rs[:m, :R],
                             start=True, stop=True)
            w = spool.tile([128, SUBRUN], f32, tag="w")
            nc.vector.tensor_tensor(out=w[:, :R], in0=em[:, :R], in1=rsrep[:, :R],
                                    op=mybir.AluOpType.mult)
            w2 = spool.tile([128, SUBRUN], f32, tag="w2")
            nc.vector.tensor_tensor(
                out=w2[:, :R], in0=w[:, :R],
                in1=xs3[:, :, D],
                op=mybir.AluOpType.mult)
            st["w2"] = w2

        def emit_C(si):
            K, m, a0, b0, indk, indkt, R = ctx_of(si)
            st = state.pop(si)
            xs3, w2 = st["xs3"], st["w2"]
            ocs = opool.tile([32, SUBRUN * D], f32, tag="ocs")
            for q in range((R + CB - 1) // CB):
                cb = min(CB, R - q * CB)
                y_eng = nc.gpsimd if Y_ON_GPSIMD else nc.vector
                Y = wpool.tile([128, CB * D], f32, tag="Y")
                w2b = w2[:, q * CB:q * CB + cb, None].to_broadcast([128, cb, D])
                y_eng.tensor_tensor(
                    out=Y[:, :cb * D].rearrange("p (b w) -> p b w", b=cb, w=D),
                    in0=xs3[:, q * CB:q * CB + cb, 0:D],
                    in1=w2b,
                    op=mybir.AluOpType.mult)
                agg = ps_a.tile([32, CB * D], f32, tag="agg")
                for h in range(0, cb * D, 512):
                    hw_ = min(512, cb * D - h)
                    nc.tensor.matmul(out=agg[:m, h:h + hw_], lhsT=indk[:, :m],
                                     rhs=Y[:, h:h + hw_], start=True, stop=True)
                nc.scalar.activation(ocs[:m, q * CB * D:q * CB * D + cb * D],
                                     agg[:m, :cb * D],
                                     mybir.ActivationFunctionType.Copy, 0.0, 1.0)
            nc.sync.dma_start(out=oarr_d[:m, a0:a0 + R * D], in_=ocs[:m, :R * D])

        n = len(subruns)
        for t in range(n + 4):
            if t < n:
                emit_A(t)
            if 2 <= t < n + 2:
                emit_B(t - 2)
            if t >= 4:
                emit_C(t - 4)

    nc.compile()
    return nc


# ----------------------------------------------------------------------------
# entry point
# ----------------------------------------------------------------------------

def kernel(x, edge_index, beta1, beta2, beta3):
    x = np.asarray(x, dtype=np.float32)
    edge_index = np.asarray(edge_index)
    betas = [float(np.asarray(b).reshape(-1)[0]) for b in (beta1, beta2, beta3)]

    loops = np.arange(N_NODES, dtype=edge_index.dtype)
    src = np.concatenate([edge_index[0], loops]).astype(np.int64)
    dst = np.concatenate([edge_index[1], loops]).astype(np.int64)

    plan = build_plan(src, dst)
    indk, indkt = host_indicators(plan)

    from concourse.bass_utils import run_bass_kernel_spmd
    key = (plan.NBLK, tuple(plan.classes))
    if key not in _NEFF_CACHE:
        _NEFF_CACHE[key] = build_nc(plan)
    nc = _NEFF_CACHE[key]

    cur = x
    for li in range(3):
        ins = host_layer_inputs(plan, cur, betas[li])
        for m in ins:
            m["indk"] = indk
            m["indkt"] = indkt
        res = run_bass_kernel_spmd(nc, ins, core_ids=list(range(N_CORES)))
        oarrs = [res.results[c]["oarr"] for c in range(N_CORES)]
        cur = host_collect_output(plan, oarrs)
    return cur



# revision 13
# speedup vs baseline: 2.7066x; 2.7066x over previous
"""AGNN (3-layer) Trainium2 kernel — transposed-payload PE-dot design.

Strategy: nodes are partitioned across 8 NeuronCores by destination; edges are
routed to the core owning the destination. Destinations are grouped into
degree classes (in-degree padded to a multiple of 4, K <= 128); a "block"
holds m = 128//K nodes' padded edge lists on 128 slots.

Per layer the host gathers each edge slot's source features into a TRANSPOSED
fp16 payload: 4 consecutive blocks are packed as a [128, 128] tile whose
partition p = 32*lane + d holds feature d of block (4g+lane)'s slots. On
device, per block b:

  dot   : C_b = xsT_b^T @ xaT_b       (PE, contraction over the 32 feature
          partitions; C_b[s, j] = x_src[s] . beta*xn_dst[j]; only m columns)
  band  : t[s] = sum_j C_b[s, j]*indk[s, j]   (DVE mult + reduce, m-wide)
  alpha = t * rnorm_src; e = exp(alpha); em = e * mask
  ss    : segment sums via indicator matmul (PE)
  W'    : em-weighted indicator (Pool)
  trans : xsT -> slot-major xs via PE transpose (fp16 PSUM) + engine copy
  agg   : aggT_b = xs_b^T @ W'_b      (PE; output [32 features, m nodes] —
          feature dim on PSUM partitions keeps the output free size at m)

The softmax denominator is NOT divided on device: the kernel returns
sum_s em*x_src and ss per node; the host divides (this folds into the
host-side inter-layer renormalization it already performs).

The device kernel is a pure streaming+compute NEFF (one compile, one run per
layer). Per-edge source rows are gathered on the host between layers (no fast
data-dependent gather on this hardware path)."""

import numpy as np
from contextlib import ExitStack

N_NODES = 100000
D = 32
N_CORES = 8
NPC = N_NODES // N_CORES
EPS = 1e-12
SUBRUN = 116                       # blocks per subrun (multiple of 4)
_NEFF_CACHE = {}

# schedule knobs (tuned via TimelineSim sweeps)
# NOTE: GPSIMD (Pool) cannot access PSUM on real hardware — only SBUF-to-SBUF
# work may go to "p" engines (the W' build, alpha/em muls).
CFG = {
    "band_eng": "v",          # band mult engine (PSUM read: DVE only)
    "pat": ("a", "v", "a", "v"),   # xsm copy engine rotation (PSUM read: a/v)
    "c_off": 4,               # stage C lag vs A
    "d_off": 7,               # stage D lag vs A
    "oarr_eng": ("a", "v"),   # oarr copy engine rotation (PSUM read: a/v)
    "alpha_eng": "p",         # alpha = t*rnorm (SBUF only -> Pool ok)
    "em_eng": "v",            # em = e*mask (SBUF only)
}


# ----------------------------------------------------------------------------
# host-side graph preprocessing (layer-invariant)
# ----------------------------------------------------------------------------

class Plan:
    pass


def build_plan(src, dst):
    """src/dst: int64 [E_tot] edge endpoints including self loops."""
    deg = np.bincount(dst, minlength=N_NODES)
    assert deg.max() <= 128, f"max in-degree {deg.max()} > 128 unsupported"
    K_of_node = 4 * np.ceil(deg / 4).astype(np.int64)
    K_of_node = np.maximum(K_of_node, 4)

    plan = Plan()
    plan.core_nodes = []
    plan.core_Ks = []
    for c in range(N_CORES):
        nodes = np.arange(c * NPC, (c + 1) * NPC)
        order = np.argsort(-K_of_node[nodes], kind="stable")
        plan.core_nodes.append(nodes[order])
        plan.core_Ks.append(K_of_node[nodes[order]])

    # class structure equalized across cores; nblk padded to a multiple of 4
    all_K = sorted(set(int(k) for c in range(N_CORES) for k in plan.core_Ks[c]),
                   reverse=True)
    plan.classes = []
    for K in all_K:
        m = 128 // K
        nblk = 0
        for c in range(N_CORES):
            nk = int((plan.core_Ks[c] == K).sum())
            nblk = max(nblk, (nk + m - 1) // m)
        nblk = ((nblk + 3) // 4) * 4
        plan.classes.append((K, m, nblk))
    plan.NBLK = sum(nblk for _, _, nblk in plan.classes)
    assert plan.NBLK % 4 == 0
    plan.NGRP = plan.NBLK // 4

    plan.class_blk_off = []
    off = 0
    for (K, m, nblk) in plan.classes:
        plan.class_blk_off.append(off)
        off += nblk
    # xaT / oarrT column offsets (m columns per block)
    plan.class_am_off = []
    off = 0
    for (K, m, nblk) in plan.classes:
        plan.class_am_off.append(off)
        off += m * nblk
    plan.AMW = off

    # indicator matrix: one m-column group per class
    plan.ind_off = {}
    plan.SUMM = 0
    seen = set()
    for (K, m, nblk) in plan.classes:
        if (K, m) not in seen:
            seen.add((K, m))
            plan.ind_off[(K, m)] = plan.SUMM
            plan.SUMM += m
    indk = np.zeros((128, plan.SUMM), dtype=np.float16)
    p = np.arange(128)
    for (K, m), ioff in plan.ind_off.items():
        sel = (p // K) < m
        indk[sel, ioff + (p // K)[sel]] = 1.0
    plan.indk = indk

    # per-core slot->src map + mask
    e_order = np.argsort(dst, kind="stable")
    src_by_dst = src[e_order]
    row_start = np.zeros(N_NODES + 1, dtype=np.int64)
    np.cumsum(deg, out=row_start[1:])

    plan.slot_src = np.zeros((N_CORES, 128, plan.NBLK), dtype=np.int32)
    plan.mask = np.zeros((N_CORES, 128, plan.NBLK), dtype=np.float16)
    plan.arr_node = np.full((N_CORES, 32, plan.NBLK), -1, dtype=np.int64)

    for c in range(N_CORES):
        Ks = plan.core_Ks[c]
        nodes_sorted = plan.core_nodes[c]
        pos = 0
        for ci, (K, m, nblk) in enumerate(plan.classes):
            nk = int((Ks == K).sum())
            cls_nodes = nodes_sorted[pos:pos + nk]
            pos += nk
            b0 = plan.class_blk_off[ci]
            for j_global in range(nk):
                b = j_global // m
                j = j_global % m
                node = cls_nodes[j_global]
                plan.arr_node[c, j, b0 + b] = node
                d0 = deg[node]
                p0 = j * K
                ss = src_by_dst[row_start[node]:row_start[node] + d0]
                plan.slot_src[c, p0:p0 + d0, b0 + b] = ss
                plan.mask[c, p0:p0 + d0, b0 + b] = 1.0

    # gather index for the transposed 4-block-packed payload:
    # xsT4[p, g*128 + s] = x[slot_src[s, 4g + p//32], p % 32]
    lane = np.arange(128) // 32                     # [128]
    plan.PMOD = (np.arange(128) % 32).astype(np.int64)
    g = np.arange(plan.NGRP)
    # XIDX[c][p, g*128+s] = slot_src[c, s, 4g + lane[p]]
    plan.XIDX = []
    for c in range(N_CORES):
        blk = (4 * g[None, :, None] + lane[:, None, None])   # [128, NGRP, 1]
        s = np.arange(128)[None, None, :]                    # [1, 1, 128]
        xi = plan.slot_src[c][s, blk]                        # [128, NGRP, 128]
        plan.XIDX.append(np.ascontiguousarray(
            xi.reshape(128, plan.NGRP * 128)))

    # subrun schedule spans (global block offset, length) — mirrors build_nc
    plan.subrun_spans = []
    for ci, (K, m, nblk) in enumerate(plan.classes):
        b = 0
        while b < nblk:
            n = min(SUBRUN, nblk - b)
            plan.subrun_spans.append((plan.class_blk_off[ci] + b, n))
            b += n

    # xaT columns: col = am_off[ci] + b_local*m + j -> node id (or -1)
    plan.col_node = []
    plan.col_j = np.zeros(plan.AMW, dtype=np.int64)
    plan.col_b = np.zeros(plan.AMW, dtype=np.int64)
    col_node = np.full((N_CORES, plan.AMW), -1, dtype=np.int64)
    for ci, (K, m, nblk) in enumerate(plan.classes):
        a0 = plan.class_am_off[ci]
        b0 = plan.class_blk_off[ci]
        for bl in range(nblk):
            cols = a0 + bl * m + np.arange(m)
            plan.col_j[cols] = np.arange(m)
            plan.col_b[cols] = b0 + bl
            col_node[:, cols] = plan.arr_node[:, :m, b0 + bl]
    plan.col_node = col_node
    return plan


def host_normalize(x):
    nrm = np.sqrt((x.astype(np.float64) ** 2).sum(axis=1))
    nrm = np.maximum(nrm, EPS).astype(np.float32)
    xn = (x / nrm[:, None]).astype(np.float32)
    return xn, nrm


def host_layer_inputs(plan, x_full, beta):
    """Per-core device inputs for one layer from the full node features."""
    xn, nrm = host_normalize(x_full)
    rnorm = (1.0 / nrm).astype(np.float32)
    xT = np.ascontiguousarray(x_full.T)             # [32, N] fp32
    xa_all = (beta * xn)                            # [N, 32]
    ins = []
    for c in range(N_CORES):
        xsT4 = xT[plan.PMOD[:, None], plan.XIDX[c]].astype(np.float16)
        cn = plan.col_node[c]
        xaT = np.zeros((32, plan.AMW), dtype=np.float16)
        valid = cn >= 0
        xaT[:, valid] = xa_all[cn[valid]].T.astype(np.float16)
        xaT = np.tile(xaT, (4, 1))
        lane_of_row = np.arange(128) // 32
        xaT *= (lane_of_row[:, None] == (plan.col_b[None, :] % 4))
        xaT = np.ascontiguousarray(xaT)
        rn = (rnorm[plan.slot_src[c]] * plan.mask[c]).astype(np.float16)
        aux = np.empty((128, 2 * plan.NBLK), dtype=np.float16)
        for (b0, R) in plan.subrun_spans:
            aux[:, 2 * b0:2 * b0 + R] = rn[:, b0:b0 + R]
            aux[:, 2 * b0 + R:2 * b0 + 2 * R] = plan.mask[c][:, b0:b0 + R]
        ins.append({
            "xsT4": np.ascontiguousarray(xsT4),
            "xaT": xaT,
            "aux": aux,
            "indk": plan.indk,
        })
    return ins


def host_collect_output(plan, oarrs, sss):
    """oarrs: per-core [32, AMW] fp16 (= sum em*x per node, transposed);
    sss: per-core [32, NBLK] fp16 segment sums. Returns full [N, D] fp32."""
    out = np.zeros((N_NODES, D), dtype=np.float32)
    drows = np.arange(32)
    for c in range(N_CORES):
        cn = plan.col_node[c]
        valid = cn >= 0
        lane = (plan.col_b[valid] % 4)
        rows = 32 * lane[None, :] + drows[:, None]      # [32, ncols]
        ssv = sss[c].astype(np.float32)[plan.col_j[valid], plan.col_b[valid]]
        vals = oarrs[c].astype(np.float32)[rows, np.where(valid)[0][None, :]] \
            / ssv[None, :]
        out[cn[valid]] = vals.T
    return out


# ----------------------------------------------------------------------------
# device kernel
# ----------------------------------------------------------------------------

def build_nc(plan):
    import concourse.bass as bass
    import concourse.tile as tile
    from concourse import bacc, mybir
    from concourse.masks import make_identity

    f16 = mybir.dt.float16
    f32 = mybir.dt.float32
    MUL = mybir.AluOpType.mult
    ADD = mybir.AluOpType.add

    nc = bacc.Bacc("TRN2", target_bir_lowering=False, debug=False)
    xsT4_d = nc.declare_dram_parameter("xsT4", [128, plan.NGRP * 128], f16, isOutput=False)
    xaT_d = nc.declare_dram_parameter("xaT", [128, plan.AMW], f16, isOutput=False)
    aux_d = nc.declare_dram_parameter("aux", [128, 2 * plan.NBLK], f16, isOutput=False)
    indk_d = nc.declare_dram_parameter("indk", [128, plan.SUMM], f16, isOutput=False)
    oarrT_d = nc.declare_dram_parameter("oarrT", [128, plan.AMW], f16, isOutput=True)
    ssout_d = nc.declare_dram_parameter("ssout", [32, plan.NBLK], f16, isOutput=True)

    # subrun schedule: (class_idx, blk_off_in_class, nblk_sub)  (all mult of 4)
    subruns = []
    for ci, (K, m, nblk) in enumerate(plan.classes):
        b = 0
        while b < nblk:
            n = min(SUBRUN, nblk - b)
            subruns.append((ci, b, n))
            b += n

    with tile.TileContext(nc) as tc, ExitStack() as ctx:
        const = ctx.enter_context(tc.tile_pool(name="const", bufs=1))
        xpool = ctx.enter_context(tc.tile_pool(name="xst", bufs=5))
        mpool = ctx.enter_context(tc.tile_pool(name="xsm", bufs=6))
        wpool = ctx.enter_context(tc.tile_pool(name="work", bufs=6))
        spool = ctx.enter_context(tc.tile_pool(name="small", bufs=8))
        ps_c = ctx.enter_context(tc.tile_pool(name="psc", bufs=3, space="PSUM"))
        ps_t = ctx.enter_context(tc.tile_pool(name="pst", bufs=2, space="PSUM"))
        ps_a = ctx.enter_context(tc.tile_pool(name="psa", bufs=2, space="PSUM"))
        ps_s = ctx.enter_context(tc.tile_pool(name="pss", bufs=1, space="PSUM"))

        # resident constants and whole-layer accumulators
        indk_sb = const.tile([128, plan.SUMM], f16)
        nc.sync.dma_start(out=indk_sb[:], in_=indk_d[:])
        aux_sb = const.tile([128, 2 * plan.NBLK], f16)
        xaT_sb = const.tile([128, plan.AMW], f16)
        third = ((plan.AMW // 3) + 511) & ~511
        for k in range(3):
            lo = k * third
            hi = min(plan.AMW, (k + 1) * third)
            if hi > lo:
                nc.sync.dma_start(out=xaT_sb[:, lo:hi], in_=xaT_d[:, lo:hi])
        ident = const.tile([128, 128], f16)
        make_identity(nc, ident[:])
        oarrT_sb = const.tile([128, plan.AMW], f16)
        ss_sb = const.tile([32, plan.NBLK], f16)

        state = {}
        copy_rr = [0]   # round-robin counter for PSUM->SBUF copy engines

        def copy_engine():
            # weighted round-robin: DVE is fastest (fp16 2x), Act mid, Pool slow
            pat = CFG["pat"]
            e = pat[copy_rr[0] % len(pat)]
            copy_rr[0] += 1
            return e

        def emit_copy(dst_ap, src_ap, eng):
            if eng == "v":
                nc.vector.tensor_scalar_mul(dst_ap, src_ap, 1.0)
            elif eng == "a":
                nc.scalar.activation(dst_ap, src_ap,
                                     mybir.ActivationFunctionType.Copy, 0.0, 1.0)
            else:
                nc.gpsimd.tensor_scalar_mul(dst_ap, src_ap, 1.0)

        def ctx_of(si):
            (ci, bo, R) = subruns[si]
            K, m, nblk = plan.classes[ci]
            b0 = plan.class_blk_off[ci] + bo          # global block offset
            a0 = plan.class_am_off[ci] + bo * m       # global am-col offset
            g0 = b0 // 4                              # global group offset
            ioff = plan.ind_off[(K, m)]
            return K, m, b0, a0, g0, ioff, R

        def emit_A(si):
            """DMA in the subrun's transposed payload."""
            K, m, b0, a0, g0, ioff, R = ctx_of(si)
            G = R // 4
            xst = xpool.tile([128, (SUBRUN // 4) * 128], f16, tag="xst")
            nc.sync.dma_start(out=xst[:, :G * 128],
                              in_=xsT4_d[:, g0 * 128:(g0 + G) * 128])
            nc.sync.dma_start(out=aux_sb[:, 2 * b0:2 * b0 + 2 * R],
                              in_=aux_d[:, 2 * b0:2 * b0 + 2 * R])
            state[si] = {"xst": xst}

        def emit_B(si):
            """Dot matmuls + band extract -> t; transposes -> slot-major xs."""
            K, m, b0, a0, g0, ioff, R = ctx_of(si)
            G = R // 4
            st = state[si]
            xst = st["xst"]
            ind = indk_sb[:, ioff:ioff + m]
            # transpose the payload to slot-major first: only depends on the DMA
            xsm = mpool.tile([128, (SUBRUN // 4) * 128], f16, tag="xsm")
            gc = 0
            while gc < G:
                gn = min(8, G - gc)
                pt = ps_t.tile([128, 8 * 128], f16, tag="T")
                for g in range(gn):
                    nc.tensor.transpose(
                        out=pt[:, g * 128:(g + 1) * 128],
                        in_=xst[:, (gc + g) * 128:(gc + g + 1) * 128],
                        identity=ident[:])
                emit_copy(xsm[:, gc * 128:(gc + gn) * 128],
                          pt[:, :gn * 128], copy_engine())
                gc += gn
            st["xsm"] = xsm
            tt = spool.tile([128, SUBRUN], f32, tag="tt")
            qc = max(4, (512 // m) & ~3)       # blocks per C-psum bank
            qb = 0
            while qb < R:
                qn = min(qc, R - qb)
                Cp = ps_c.tile([128, 512], f32, tag="C")
                for b4 in range(qn // 4):
                    gg = (qb + b4 * 4) // 4
                    nc.tensor.matmul(
                        out=Cp[:, b4 * 4 * m:(b4 + 1) * 4 * m],
                        lhsT=xst[:, gg * 128:(gg + 1) * 128],
                        rhs=xaT_sb[:, a0 + (qb + b4 * 4) * m:
                                   a0 + (qb + (b4 + 1) * 4) * m],
                        start=True, stop=True)
                bp = wpool.tile([128, 512], f16, tag="bp")
                if CFG["band_eng"] == "v":
                    beng = nc.vector
                else:
                    beng = nc.vector if (qb // qc) % 2 == 0 else nc.gpsimd
                beng.tensor_tensor(
                    out=bp[:, :qn * m].rearrange("p (b j) -> p b j", b=qn, j=m),
                    in0=Cp[:, :qn * m].rearrange("p (b j) -> p b j", b=qn, j=m),
                    in1=ind[:, None, :].to_broadcast([128, qn, m]),
                    op=MUL)
                nc.vector.tensor_reduce(
                    out=tt[:, qb:qb + qn],
                    in_=bp[:, :qn * m].rearrange("p (b j) -> p b j", b=qn, j=m),
                    axis=mybir.AxisListType.X, op=ADD)
                qb += qn
            st["tt"] = tt

        def emit_C(si):
            """alpha -> exp -> em; segment sums; em-weighted indicator."""
            K, m, b0, a0, g0, ioff, R = ctx_of(si)
            st = state[si]
            tt = st.pop("tt")
            ind = indk_sb[:, ioff:ioff + m]
            alpha = spool.tile([128, SUBRUN], f32, tag="alpha")
            aeng = {"v": nc.vector, "p": nc.gpsimd}[CFG["alpha_eng"]]
            aeng.tensor_tensor(out=alpha[:, :R], in0=tt[:, :R],
                               in1=aux_sb[:, 2 * b0:2 * b0 + R], op=MUL)
            e = spool.tile([128, SUBRUN], f16, tag="e")
            nc.scalar.activation(e[:, :R], alpha[:, :R],
                                 mybir.ActivationFunctionType.Exp, 0.0, 1.0)
            em = spool.tile([128, SUBRUN], f16, tag="em")
            meng = {"v": nc.vector, "p": nc.gpsimd}[CFG["em_eng"]]
            meng.tensor_tensor(out=em[:, :R], in0=e[:, :R],
                               in1=aux_sb[:, 2 * b0 + R:2 * b0 + 2 * R],
                               op=MUL)
            pss = ps_s.tile([32, 128], f32, tag="ss")
            nc.tensor.matmul(out=pss[:m, :R], lhsT=ind, rhs=em[:, :R],
                             start=True, stop=True)
            nc.scalar.activation(ss_sb[0:m, b0:b0 + R], pss[:m, :R],
                                 mybir.ActivationFunctionType.Copy, 0.0, 1.0)
            wp = wpool.tile([128, SUBRUN * 32], f16, tag="wp")
            nc.gpsimd.tensor_tensor(
                out=wp[:, :R * m].rearrange("p (b j) -> p b j", b=R, j=m),
                in0=em[:, :R, None].to_broadcast([128, R, m]),
                in1=ind[:, None, :].to_broadcast([128, R, m]),
                op=MUL)
            st["wp"] = wp

        def emit_D(si):
            """Per-block aggregation matmuls (feature dim on PSUM partitions)."""
            K, m, b0, a0, g0, ioff, R = ctx_of(si)
            st = state.pop(si)
            xsm, wp = st["xsm"], st["wp"]
            qc = max(4, (512 // m) & ~3)
            qb = 0
            while qb < R:
                qn = min(qc, R - qb)
                Ap = ps_a.tile([128, 512], f32, tag="A")
                for b4 in range(qn // 4):
                    gg = (qb + b4 * 4) // 4
                    nc.tensor.matmul(
                        out=Ap[:, b4 * 4 * m:(b4 + 1) * 4 * m],
                        lhsT=xsm[:, gg * 128:(gg + 1) * 128],
                        rhs=wp[:, (qb + b4 * 4) * m:(qb + (b4 + 1) * 4) * m],
                        start=True, stop=True)
                oeng = CFG["oarr_eng"][(qb // qc) % len(CFG["oarr_eng"])]
                emit_copy(oarrT_sb[:, a0 + qb * m:a0 + (qb + qn) * m],
                          Ap[:, :qn * m], oeng)
                qb += qn
            nc.scalar.dma_start(out=oarrT_d[:, a0:a0 + R * m],
                                in_=oarrT_sb[:, a0:a0 + R * m])

        n = len(subruns)
        co, do = CFG["c_off"], CFG["d_off"]
        for t in range(n + do):
            if t < n:
                emit_A(t)
            if do <= t:
                emit_D(t - do)
            if co <= t < n + co:
                emit_C(t - co)
            if 2 <= t < n + 2:
                emit_B(t - 2)

        nc.sync.dma_start(out=ssout_d[:], in_=ss_sb[:])

    nc.compile()
    return nc


# ----------------------------------------------------------------------------
# entry point
# ----------------------------------------------------------------------------

def kernel(x, edge_index, beta1, beta2, beta3):
    x = np.asarray(x, dtype=np.float32)
    edge_index = np.asarray(edge_index)
    betas = [float(np.asarray(b).reshape(-1)[0]) for b in (beta1, beta2, beta3)]

    loops = np.arange(N_NODES, dtype=edge_index.dtype)
    src = np.concatenate([edge_index[0], loops]).astype(np.int64)
    dst = np.concatenate([edge_index[1], loops]).astype(np.int64)

    plan = build_plan(src, dst)

    from concourse.bass_utils import run_bass_kernel_spmd
    key = (plan.NBLK, tuple(plan.classes))
    if key not in _NEFF_CACHE:
        _NEFF_CACHE[key] = build_nc(plan)
    nc = _NEFF_CACHE[key]

    cur = x
    for li in range(3):
        ins = host_layer_inputs(plan, cur, betas[li])
        res = run_bass_kernel_spmd(nc, ins, core_ids=list(range(N_CORES)))
        oarrs = [res.results[c]["oarrT"] for c in range(N_CORES)]
        sss = [res.results[c]["ssout"] for c in range(N_CORES)]
        cur = host_collect_output(plan, oarrs, sss)
    return cur


# revision 17
# speedup vs baseline: 2.9360x; 1.0847x over previous
"""AGNN (3-layer) Trainium2 kernel — transposed-payload PE-dot design.

Strategy: nodes are partitioned across 8 NeuronCores by destination; edges are
routed to the core owning the destination. Destinations are grouped into
degree classes (in-degree padded to a multiple of 4, K <= 128); a "block"
holds m = 128//K nodes' padded edge lists on 128 slots.

Per layer the host gathers each edge slot's source features into a TRANSPOSED
fp16 payload: 4 consecutive blocks are packed as a [128, 128] tile whose
partition p = 32*lane + d holds feature d of block (4g+lane)'s slots. On
device, per block b:

  dot   : C_b = xsT_b^T @ xaT_b       (PE, contraction over the 32 feature
          partitions; C_b[s, j] = x_src[s] . beta*xn_dst[j]; only m columns)
  band  : t[s] = sum_j C_b[s, j]*indk[s, j]   (DVE mult + reduce, m-wide)
  alpha = t * rnorm_src; e = exp(alpha); em = e * mask
  ss    : segment sums via indicator matmul (PE)
  W'    : em-weighted indicator (Pool)
  trans : xsT -> slot-major xs via PE transpose (fp16 PSUM) + engine copy
  agg   : aggT_b = xs_b^T @ W'_b      (PE; output [32 features, m nodes] —
          feature dim on PSUM partitions keeps the output free size at m)

The softmax denominator is NOT divided on device: the kernel returns
sum_s em*x_src and ss per node; the host divides (this folds into the
host-side inter-layer renormalization it already performs).

The device kernel is a pure streaming+compute NEFF (one compile, one run per
layer). Per-edge source rows are gathered on the host between layers (no fast
data-dependent gather on this hardware path)."""

import numpy as np
from contextlib import ExitStack

N_NODES = 100000
D = 32
N_CORES = 8
NPC = N_NODES // N_CORES
EPS = 1e-12
SUBRUN = 116                       # blocks per subrun (multiple of 4)
_NEFF_CACHE = {}

# schedule knobs (tuned via TimelineSim sweeps)
# NOTE: GPSIMD (Pool) cannot access PSUM on real hardware — only SBUF-to-SBUF
# work may go to "p" engines (the W' build, alpha/em muls).
CFG = {
    "band_eng": "v",          # band mult engine (PSUM read: DVE only)
    "pat": ("a", "v", "a", "v"),   # xsm copy engine rotation (PSUM read: a/v)
    "c_off": 4,               # stage C lag vs A
    "d_off": 6,               # stage D lag vs A
    "oarr_eng": ("a", "v"),   # oarr copy engine rotation (PSUM read: a/v)
    "alpha_eng": "p",         # alpha = t*rnorm (SBUF only -> Pool ok)
    "em_eng": "v",            # em = e*mask (SBUF only)
    "psc": 2, "pst": 3, "psa": 2, "pss": 1,   # PSUM pool bufs
}


# ----------------------------------------------------------------------------
# host-side graph preprocessing (layer-invariant)
# ----------------------------------------------------------------------------

class Plan:
    pass


def build_plan(src, dst):
    """src/dst: int64 [E_tot] edge endpoints including self loops."""
    deg = np.bincount(dst, minlength=N_NODES)
    assert deg.max() <= 128, f"max in-degree {deg.max()} > 128 unsupported"
    K_of_node = 4 * np.ceil(deg / 4).astype(np.int64)
    K_of_node = np.maximum(K_of_node, 4)

    plan = Plan()
    plan.core_nodes = []
    plan.core_Ks = []
    for c in range(N_CORES):
        nodes = np.arange(c * NPC, (c + 1) * NPC)
        order = np.argsort(-K_of_node[nodes], kind="stable")
        plan.core_nodes.append(nodes[order])
        plan.core_Ks.append(K_of_node[nodes[order]])

    # class structure equalized across cores; nblk padded to a multiple of 4
    all_K = sorted(set(int(k) for c in range(N_CORES) for k in plan.core_Ks[c]),
                   reverse=True)
    plan.classes = []
    for K in all_K:
        m = 128 // K
        nblk = 0
        for c in range(N_CORES):
            nk = int((plan.core_Ks[c] == K).sum())
            nblk = max(nblk, (nk + m - 1) // m)
        nblk = ((nblk + 3) // 4) * 4
        plan.classes.append((K, m, nblk))
    plan.NBLK = sum(nblk for _, _, nblk in plan.classes)
    assert plan.NBLK % 4 == 0
    plan.NGRP = plan.NBLK // 4

    plan.class_blk_off = []
    off = 0
    for (K, m, nblk) in plan.classes:
        plan.class_blk_off.append(off)
        off += nblk
    # xaT / oarrT column offsets (m columns per block)
    plan.class_am_off = []
    off = 0
    for (K, m, nblk) in plan.classes:
        plan.class_am_off.append(off)
        off += m * nblk
    plan.AMW = off

    # indicator matrix: one m-column group per class
    plan.ind_off = {}
    plan.SUMM = 0
    seen = set()
    for (K, m, nblk) in plan.classes:
        if (K, m) not in seen:
            seen.add((K, m))
            plan.ind_off[(K, m)] = plan.SUMM
            plan.SUMM += m
    indk = np.zeros((128, plan.SUMM), dtype=np.float16)
    p = np.arange(128)
    for (K, m), ioff in plan.ind_off.items():
        sel = (p // K) < m
        indk[sel, ioff + (p // K)[sel]] = 1.0
    plan.indk = indk

    # per-core slot->src map + mask
    e_order = np.argsort(dst, kind="stable")
    src_by_dst = src[e_order]
    row_start = np.zeros(N_NODES + 1, dtype=np.int64)
    np.cumsum(deg, out=row_start[1:])

    plan.slot_src = np.zeros((N_CORES, 128, plan.NBLK), dtype=np.int32)
    plan.mask = np.zeros((N_CORES, 128, plan.NBLK), dtype=np.float16)
    plan.arr_node = np.full((N_CORES, 32, plan.NBLK), -1, dtype=np.int64)

    for c in range(N_CORES):
        Ks = plan.core_Ks[c]
        nodes_sorted = plan.core_nodes[c]
        pos = 0
        for ci, (K, m, nblk) in enumerate(plan.classes):
            nk = int((Ks == K).sum())
            cls_nodes = nodes_sorted[pos:pos + nk]
            pos += nk
            b0 = plan.class_blk_off[ci]
            for j_global in range(nk):
                b = j_global // m
                j = j_global % m
                node = cls_nodes[j_global]
                plan.arr_node[c, j, b0 + b] = node
                d0 = deg[node]
                p0 = j * K
                ss = src_by_dst[row_start[node]:row_start[node] + d0]
                plan.slot_src[c, p0:p0 + d0, b0 + b] = ss
                plan.mask[c, p0:p0 + d0, b0 + b] = 1.0

    # gather index for the transposed 4-block-packed payload:
    # xsT4[p, g*128 + s] = x[slot_src[s, 4g + p//32], p % 32]
    lane = np.arange(128) // 32                     # [128]
    plan.PMOD = (np.arange(128) % 32).astype(np.int64)
    g = np.arange(plan.NGRP)
    # XIDX[c][p, g*128+s] = slot_src[c, s, 4g + lane[p]]
    plan.XIDX = []
    for c in range(N_CORES):
        blk = (4 * g[None, :, None] + lane[:, None, None])   # [128, NGRP, 1]
        s = np.arange(128)[None, None, :]                    # [1, 1, 128]
        xi = plan.slot_src[c][s, blk]                        # [128, NGRP, 128]
        plan.XIDX.append(np.ascontiguousarray(
            xi.reshape(128, plan.NGRP * 128)))

    # subrun schedule spans (global block offset, length) — mirrors build_nc
    plan.subrun_spans = []
    for ci, (K, m, nblk) in enumerate(plan.classes):
        b = 0
        while b < nblk:
            n = min(SUBRUN, nblk - b)
            plan.subrun_spans.append((plan.class_blk_off[ci] + b, n))
            b += n

    # xaT columns: col = am_off[ci] + b_local*m + j -> node id (or -1)
    plan.col_node = []
    plan.col_j = np.zeros(plan.AMW, dtype=np.int64)
    plan.col_b = np.zeros(plan.AMW, dtype=np.int64)
    col_node = np.full((N_CORES, plan.AMW), -1, dtype=np.int64)
    for ci, (K, m, nblk) in enumerate(plan.classes):
        a0 = plan.class_am_off[ci]
        b0 = plan.class_blk_off[ci]
        for bl in range(nblk):
            cols = a0 + bl * m + np.arange(m)
            plan.col_j[cols] = np.arange(m)
            plan.col_b[cols] = b0 + bl
            col_node[:, cols] = plan.arr_node[:, :m, b0 + bl]
    plan.col_node = col_node
    return plan


def host_normalize(x):
    nrm = np.sqrt((x.astype(np.float64) ** 2).sum(axis=1))
    nrm = np.maximum(nrm, EPS).astype(np.float32)
    xn = (x / nrm[:, None]).astype(np.float32)
    return xn, nrm


def host_layer_inputs(plan, x_full, beta):
    """Per-core device inputs for one layer from the full node features."""
    xn, nrm = host_normalize(x_full)
    rnorm = (1.0 / nrm).astype(np.float32)
    xT = np.ascontiguousarray(x_full.T)             # [32, N] fp32
    xa_all = (beta * xn)                            # [N, 32]
    ins = []
    for c in range(N_CORES):
        xsT4 = xT[plan.PMOD[:, None], plan.XIDX[c]].astype(np.float16)
        cn = plan.col_node[c]
        xaT = np.zeros((32, plan.AMW), dtype=np.float16)
        valid = cn >= 0
        xaT[:, valid] = xa_all[cn[valid]].T.astype(np.float16)
        xaT = np.tile(xaT, (4, 1))
        lane_of_row = np.arange(128) // 32
        xaT *= (lane_of_row[:, None] == (plan.col_b[None, :] % 4))
        xaT = np.ascontiguousarray(xaT)
        rn = (rnorm[plan.slot_src[c]] * plan.mask[c]).astype(np.float16)
        aux = np.empty((128, 2 * plan.NBLK), dtype=np.float16)
        for (b0, R) in plan.subrun_spans:
            aux[:, 2 * b0:2 * b0 + R] = rn[:, b0:b0 + R]
            aux[:, 2 * b0 + R:2 * b0 + 2 * R] = plan.mask[c][:, b0:b0 + R]
        ins.append({
            "xsT4": np.ascontiguousarray(xsT4),
            "xaT": xaT,
            "aux": aux,
            "indk": plan.indk,
        })
    return ins


def host_collect_output(plan, oarrs, sss):
    """oarrs: per-core [32, AMW] fp16 (= sum em*x per node, transposed);
    sss: per-core [32, NBLK] fp16 segment sums. Returns full [N, D] fp32."""
    out = np.zeros((N_NODES, D), dtype=np.float32)
    drows = np.arange(32)
    for c in range(N_CORES):
        cn = plan.col_node[c]
        valid = cn >= 0
        lane = (plan.col_b[valid] % 4)
        rows = 32 * lane[None, :] + drows[:, None]      # [32, ncols]
        ssv = sss[c].astype(np.float32)[plan.col_j[valid], plan.col_b[valid]]
        vals = oarrs[c].astype(np.float32)[rows, np.where(valid)[0][None, :]] \
            / ssv[None, :]
        out[cn[valid]] = vals.T
    return out


# ----------------------------------------------------------------------------
# device kernel
# ----------------------------------------------------------------------------

def build_nc(plan):
    import concourse.bass as bass
    import concourse.tile as tile
    from concourse import bacc, mybir
    from concourse.masks import make_identity

    f16 = mybir.dt.float16
    f32 = mybir.dt.float32
    MUL = mybir.AluOpType.mult
    ADD = mybir.AluOpType.add

    nc = bacc.Bacc("TRN2", target_bir_lowering=False, debug=False)
    xsT4_d = nc.declare_dram_parameter("xsT4", [128, plan.NGRP * 128], f16, isOutput=False)
    xaT_d = nc.declare_dram_parameter("xaT", [128, plan.AMW], f16, isOutput=False)
    aux_d = nc.declare_dram_parameter("aux", [128, 2 * plan.NBLK], f16, isOutput=False)
    indk_d = nc.declare_dram_parameter("indk", [128, plan.SUMM], f16, isOutput=False)
    oarrT_d = nc.declare_dram_parameter("oarrT", [128, plan.AMW], f16, isOutput=True)
    ssout_d = nc.declare_dram_parameter("ssout", [32, plan.NBLK], f16, isOutput=True)

    # subrun schedule: (class_idx, blk_off_in_class, nblk_sub)  (all mult of 4)
    subruns = []
    for ci, (K, m, nblk) in enumerate(plan.classes):
        b = 0
        while b < nblk:
            n = min(SUBRUN, nblk - b)
            subruns.append((ci, b, n))
            b += n

    with tile.TileContext(nc) as tc, ExitStack() as ctx:
        const = ctx.enter_context(tc.tile_pool(name="const", bufs=1))
        xpool = ctx.enter_context(tc.tile_pool(name="xst", bufs=5))
        mpool = ctx.enter_context(tc.tile_pool(name="xsm", bufs=6))
        wpool = ctx.enter_context(tc.tile_pool(name="work", bufs=6))
        spool = ctx.enter_context(tc.tile_pool(name="small", bufs=8))
        ps_c = ctx.enter_context(tc.tile_pool(name="psc", bufs=CFG["psc"], space="PSUM"))
        ps_t = ctx.enter_context(tc.tile_pool(name="pst", bufs=CFG["pst"], space="PSUM"))
        ps_a = ctx.enter_context(tc.tile_pool(name="psa", bufs=CFG["psa"], space="PSUM"))
        ps_s = ctx.enter_context(tc.tile_pool(name="pss", bufs=CFG["pss"], space="PSUM"))

        # resident constants and whole-layer accumulators
        indk_sb = const.tile([128, plan.SUMM], f16)
        nc.sync.dma_start(out=indk_sb[:], in_=indk_d[:])
        aux_sb = const.tile([128, 2 * plan.NBLK], f16)
        xaT_sb = const.tile([128, plan.AMW], f16)
        third = ((plan.AMW // 3) + 511) & ~511

        def emit_xabd_slice(k):
            lo = k * third
            hi = min(plan.AMW, (k + 1) * third)
            if hi > lo:
                nc.sync.dma_start(out=xaT_sb[:, lo:hi], in_=xaT_d[:, lo:hi])
        emit_xabd_slice(0)
        ident = const.tile([128, 128], f16)
        make_identity(nc, ident[:])
        oarrT_sb = const.tile([128, plan.AMW], f16)
        ss_sb = const.tile([32, plan.NBLK], f16)

        state = {}
        copy_rr = [0]   # round-robin counter for PSUM->SBUF copy engines

        def copy_engine():
            # weighted round-robin: DVE is fastest (fp16 2x), Act mid, Pool slow
            pat = CFG["pat"]
            e = pat[copy_rr[0] % len(pat)]
            copy_rr[0] += 1
            return e

        def emit_copy(dst_ap, src_ap, eng):
            if eng == "v":
                nc.vector.tensor_scalar_mul(dst_ap, src_ap, 1.0)
            elif eng == "a":
                nc.scalar.activation(dst_ap, src_ap,
                                     mybir.ActivationFunctionType.Copy, 0.0, 1.0)
            else:
                nc.gpsimd.tensor_scalar_mul(dst_ap, src_ap, 1.0)

        def ctx_of(si):
            (ci, bo, R) = subruns[si]
            K, m, nblk = plan.classes[ci]
            b0 = plan.class_blk_off[ci] + bo          # global block offset
            a0 = plan.class_am_off[ci] + bo * m       # global am-col offset
            g0 = b0 // 4                              # global group offset
            ioff = plan.ind_off[(K, m)]
            return K, m, b0, a0, g0, ioff, R

        def emit_A(si):
            """DMA in the subrun's transposed payload."""
            K, m, b0, a0, g0, ioff, R = ctx_of(si)
            G = R // 4
            xst = xpool.tile([128, (SUBRUN // 4) * 128], f16, tag="xst")
            nc.sync.dma_start(out=xst[:, :G * 128],
                              in_=xsT4_d[:, g0 * 128:(g0 + G) * 128])
            nc.sync.dma_start(out=aux_sb[:, 2 * b0:2 * b0 + 2 * R],
                              in_=aux_d[:, 2 * b0:2 * b0 + 2 * R])
            state[si] = {"xst": xst}

        def emit_B(si):
            """Dot matmuls + band extract -> t; transposes -> slot-major xs."""
            K, m, b0, a0, g0, ioff, R = ctx_of(si)
            G = R // 4
            st = state[si]
            xst = st["xst"]
            ind = indk_sb[:, ioff:ioff + m]
            # transpose the payload to slot-major first: only depends on the DMA
            xsm = mpool.tile([128, (SUBRUN // 4) * 128], f16, tag="xsm")
            gc = 0
            while gc < G:
                gn = min(8, G - gc)
                pt = ps_t.tile([128, 8 * 128], f16, tag="T")
                for g in range(gn):
                    nc.tensor.transpose(
                        out=pt[:, g * 128:(g + 1) * 128],
                        in_=xst[:, (gc + g) * 128:(gc + g + 1) * 128],
                        identity=ident[:])
                emit_copy(xsm[:, gc * 128:(gc + gn) * 128],
                          pt[:, :gn * 128], copy_engine())
                gc += gn
            st["xsm"] = xsm
            tt = spool.tile([128, SUBRUN], f32, tag="tt")
            qc = max(4, (512 // m) & ~3)       # blocks per C-psum bank
            qb = 0
            while qb < R:
                qn = min(qc, R - qb)
                Cp = ps_c.tile([128, 512], f32, tag="C")
                for b4 in range(qn // 4):
                    gg = (qb + b4 * 4) // 4
                    nc.tensor.matmul(
                        out=Cp[:, b4 * 4 * m:(b4 + 1) * 4 * m],
                        lhsT=xst[:, gg * 128:(gg + 1) * 128],
                        rhs=xaT_sb[:, a0 + (qb + b4 * 4) * m:
                                   a0 + (qb + (b4 + 1) * 4) * m],
                        start=True, stop=True)
                bp = wpool.tile([128, 512], f16, tag="bp")
                if CFG["band_eng"] == "v":
                    beng = nc.vector
                else:
                    beng = nc.vector if (qb // qc) % 2 == 0 else nc.gpsimd
                beng.tensor_tensor(
                    out=bp[:, :qn * m].rearrange("p (b j) -> p b j", b=qn, j=m),
                    in0=Cp[:, :qn * m].rearrange("p (b j) -> p b j", b=qn, j=m),
                    in1=ind[:, None, :].to_broadcast([128, qn, m]),
                    op=MUL)
                nc.vector.tensor_reduce(
                    out=tt[:, qb:qb + qn],
                    in_=bp[:, :qn * m].rearrange("p (b j) -> p b j", b=qn, j=m),
                    axis=mybir.AxisListType.X, op=ADD)
                qb += qn
            st["tt"] = tt

        def emit_C(si):
            """alpha -> exp -> em; segment sums; em-weighted indicator."""
            K, m, b0, a0, g0, ioff, R = ctx_of(si)
            st = state[si]
            tt = st.pop("tt")
            ind = indk_sb[:, ioff:ioff + m]
            alpha = spool.tile([128, SUBRUN], f32, tag="alpha")
            aeng = {"v": nc.vector, "p": nc.gpsimd}[CFG["alpha_eng"]]
            aeng.tensor_tensor(out=alpha[:, :R], in0=tt[:, :R],
                               in1=aux_sb[:, 2 * b0:2 * b0 + R], op=MUL)
            e = spool.tile([128, SUBRUN], f16, tag="e")
            nc.scalar.activation(e[:, :R], alpha[:, :R],
                                 mybir.ActivationFunctionType.Exp, 0.0, 1.0)
            em = spool.tile([128, SUBRUN], f16, tag="em")
            meng = {"v": nc.vector, "p": nc.gpsimd}[CFG["em_eng"]]
            meng.tensor_tensor(out=em[:, :R], in0=e[:, :R],
                               in1=aux_sb[:, 2 * b0 + R:2 * b0 + 2 * R],
                               op=MUL)
            pss = ps_s.tile([32, 128], f32, tag="ss")
            nc.tensor.matmul(out=pss[:m, :R], lhsT=ind, rhs=em[:, :R],
                             start=True, stop=True)
            nc.scalar.activation(ss_sb[0:m, b0:b0 + R], pss[:m, :R],
                                 mybir.ActivationFunctionType.Copy, 0.0, 1.0)
            if CFG.get("ss_chunked"):
                nc.scalar.dma_start(out=ssout_d[:, b0:b0 + R],
                                    in_=ss_sb[:, b0:b0 + R])
            wp = wpool.tile([128, SUBRUN * 32], f16, tag="wp")
            nc.gpsimd.tensor_tensor(
                out=wp[:, :R * m].rearrange("p (b j) -> p b j", b=R, j=m),
                in0=em[:, :R, None].to_broadcast([128, R, m]),
                in1=ind[:, None, :].to_broadcast([128, R, m]),
                op=MUL)
            st["wp"] = wp

        def emit_D(si):
            """Per-block aggregation matmuls (feature dim on PSUM partitions)."""
            K, m, b0, a0, g0, ioff, R = ctx_of(si)
            st = state.pop(si)
            xsm, wp = st["xsm"], st["wp"]
            qc = max(4, (512 // m) & ~3)
            qb = 0
            while qb < R:
                qn = min(qc, R - qb)
                Ap = ps_a.tile([128, 512], f32, tag="A")
                for b4 in range(qn // 4):
                    gg = (qb + b4 * 4) // 4
                    nc.tensor.matmul(
                        out=Ap[:, b4 * 4 * m:(b4 + 1) * 4 * m],
                        lhsT=xsm[:, gg * 128:(gg + 1) * 128],
                        rhs=wp[:, (qb + b4 * 4) * m:(qb + (b4 + 1) * 4) * m],
                        start=True, stop=True)
                oeng = CFG["oarr_eng"][(qb // qc) % len(CFG["oarr_eng"])]
                emit_copy(oarrT_sb[:, a0 + qb * m:a0 + (qb + qn) * m],
                          Ap[:, :qn * m], oeng)
                qb += qn
            nc.scalar.dma_start(out=oarrT_d[:, a0:a0 + R * m],
                                in_=oarrT_sb[:, a0:a0 + R * m])

        n = len(subruns)
        co, do = CFG["c_off"], CFG["d_off"]
        for t in range(n + do):
            if t < n:
                emit_A(t)
            if t in (0, 1):
                emit_xabd_slice(t + 1)
            if do <= t:
                emit_D(t - do)
            if co <= t < n + co:
                emit_C(t - co)
            if 2 <= t < n + 2:
                emit_B(t - 2)

        if not CFG.get("ss_chunked"):
            nc.sync.dma_start(out=ssout_d[:], in_=ss_sb[:])

    nc.compile()
    return nc


# ----------------------------------------------------------------------------
# entry point
# ----------------------------------------------------------------------------

def kernel(x, edge_index, beta1, beta2, beta3):
    x = np.asarray(x, dtype=np.float32)
    edge_index = np.asarray(edge_index)
    betas = [float(np.asarray(b).reshape(-1)[0]) for b in (beta1, beta2, beta3)]

    loops = np.arange(N_NODES, dtype=edge_index.dtype)
    src = np.concatenate([edge_index[0], loops]).astype(np.int64)
    dst = np.concatenate([edge_index[1], loops]).astype(np.int64)

    plan = build_plan(src, dst)

    from concourse.bass_utils import run_bass_kernel_spmd
    key = (plan.NBLK, tuple(plan.classes))
    if key not in _NEFF_CACHE:
        _NEFF_CACHE[key] = build_nc(plan)
    nc = _NEFF_CACHE[key]

    cur = x
    for li in range(3):
        ins = host_layer_inputs(plan, cur, betas[li])
        res = run_bass_kernel_spmd(nc, ins, core_ids=list(range(N_CORES)))
        oarrs = [res.results[c]["oarrT"] for c in range(N_CORES)]
        sss = [res.results[c]["ssout"] for c in range(N_CORES)]
        cur = host_collect_output(plan, oarrs, sss)
    return cur
